# revision 1
# baseline (speedup 1.0000x reference)
"""Grouped linear (MoE grouped GEMM) on 8 TRN2 NeuronCores.

Reference computes out = ragged_dot(x, weight.swap(1,2), group_lens):
tokens are pre-sorted into G=16 contiguous groups; tokens of group g are
multiplied by weight[g].T.

Strategy (token-parallel with host-side dispatch):
  - Host splits the token stream into 512-token tiles that never cross a
    group boundary (boundary tiles are zero-padded).  Every tile is a
    dense [512, 1024] x [1024, 1024] GEMM against one expert's weight.
  - Tiles are distributed evenly over the 8 cores; every core runs the
    identical program on T tiles (SPMD), with per-core data:
      xt[t]: [128(p), 8(k), 512(n)]  = x[tok n, in k*128+p]   (bf16)
      wt[t]: [128(p), 8(k), 1024(o)] = weight[g_t, o, k*128+p] (bf16)
      ot[t]: [128(j), 8(ob), 512(n)] = out[tok n, ob*128+j]    (fp32)
    All three are laid out so each DMA is contiguous per partition row.
  - On-chip: for each of 8 out-blocks, accumulate 8 k-step matmuls
    (bf16 x bf16 -> fp32 PSUM), copy PSUM->SBUF, DMA out.
  - Host scatters tile outputs back into the full [32768, 1024] fp32.
"""

import numpy as np
import ml_dtypes

import concourse.bass as bass
import concourse.tile as tile
from concourse import bacc, mybir
from concourse.bass_utils import run_bass_kernel_spmd

G, NTOK, DIN, DOUT = 16, 32768, 1024, 1024
NCORES = 8
TT = 512            # tokens per tile
KT = DIN // 128     # 8 contraction sub-tiles
OB = DOUT // 128    # 8 output blocks

_NC_CACHE: dict = {}


def _build(T: int):
    """Build + schedule the SPMD program for T tiles per core."""
    key = T
    if key in _NC_CACHE:
        return _NC_CACHE[key]
    dt_in = mybir.dt.bfloat16
    dt_out = mybir.dt.float32

    nc = bacc.Bacc(None, target_bir_lowering=False)
    xt = nc.declare_dram_parameter("xt", [T, 128, KT, TT], dt_in, isOutput=False)
    wt = nc.declare_dram_parameter("wt", [T, 128, KT, DOUT], dt_in, isOutput=False)
    ot = nc.declare_dram_parameter("ot", [T, 128, OB, TT], dt_out, isOutput=True)

    with tile.TileContext(nc) as tc:
        with (
            tc.tile_pool(name="wp", bufs=2) as wpool,
            tc.tile_pool(name="xp", bufs=3) as xpool,
            tc.tile_pool(name="op", bufs=3) as opool,
            tc.tile_pool(name="ps", bufs=8, space=bass.MemorySpace.PSUM) as pspool,
        ):
            for t in range(T):
                wsb = wpool.tile([128, KT, DOUT], dt_in)
                xsb = xpool.tile([128, KT, TT], dt_in)
                osb = opool.tile([128, OB, TT], dt_out)
                nc.sync.dma_start(wsb[:], wt[t])
                nc.sync.dma_start(xsb[:], xt[t])
                for o in range(OB):
                    ps = pspool.tile([128, TT], mybir.dt.float32)
                    for k in range(KT):
                        nc.tensor.matmul(
                            ps[:],
                            wsb[:, k, o * 128 : (o + 1) * 128],
                            xsb[:, k, :],
                            start=(k == 0),
                            stop=(k == KT - 1),
                        )
                    nc.vector.tensor_copy(osb[:, o, :], ps[:])
                nc.sync.dma_start(ot[t], osb[:])

    nc.compile()
    _NC_CACHE[key] = nc
    return nc


def _plan(group_lens):
    """Split the token stream into <=512-token single-group tiles."""
    edges = np.concatenate([[0], np.cumsum(np.asarray(group_lens, np.int64))])
    tiles = []  # (group, tok_start, ntok)
    for g in range(G):
        s, e = int(edges[g]), int(edges[g + 1])
        while s < e:
            n = min(TT, e - s)
            tiles.append((g, s, n))
            s += n
    T = (len(tiles) + NCORES - 1) // NCORES
    while len(tiles) < T * NCORES:
        tiles.append((0, 0, 0))  # dummy: zero x -> zero out, dropped on gather
    return tiles, T


def _prep_inputs(x, weight, tiles, T):
    xbf = x.astype(ml_dtypes.bfloat16)
    # wp[g][p, k, o] = weight[g, o, k*128+p]
    wp = np.ascontiguousarray(
        weight.reshape(G, DOUT, KT, 128).transpose(0, 3, 2, 1)
    ).astype(ml_dtypes.bfloat16)
    in_maps = []
    for c in range(NCORES):
        ctiles = tiles[c * T : (c + 1) * T]
        gids = np.array([g for g, _, _ in ctiles], np.int64)
        wtc = np.ascontiguousarray(wp[gids])  # [T, 128, KT, DOUT]
        xtc = np.zeros((T, 128, KT, TT), ml_dtypes.bfloat16)
        for ti, (_, s, n) in enumerate(ctiles):
            if n == 0:
                continue
            b = np.zeros((TT, DIN), ml_dtypes.bfloat16)
            b[:n] = xbf[s : s + n]
            # xtc[ti][p, k, n] = b[n, k*128+p]
            xtc[ti] = b.reshape(TT, KT, 128).transpose(2, 1, 0)
        in_maps.append({"xt": xtc, "wt": wtc})
    return in_maps


def _gather_out(results, tiles, T):
    out = np.empty((NTOK, DOUT), np.float32)
    for c in range(NCORES):
        otc = np.asarray(results[c]["ot"])  # [T, 128, OB, TT] fp32
        for ti, (_, s, n) in enumerate(tiles[c * T : (c + 1) * T]):
            if n == 0:
                continue
            # out[s+n', ob*128+j] = otc[ti][j, ob, n']
            out[s : s + n] = (
                otc[ti].transpose(2, 1, 0).reshape(TT, DOUT)[:n]
            )
    return out


def kernel(x, weight, group_lens):
    x = np.ascontiguousarray(np.asarray(x))
    weight = np.ascontiguousarray(np.asarray(weight))
    tiles, T = _plan(group_lens)
    nc = _build(T)
    in_maps = _prep_inputs(x, weight, tiles, T)
    res = run_bass_kernel_spmd(nc, in_maps, list(range(NCORES)))
    return _gather_out(res.results, tiles, T)


def _numpy_ref(x, weight, group_lens):
    edges = np.concatenate([[0], np.cumsum(np.asarray(group_lens, np.int64))])
    out = np.empty((x.shape[0], weight.shape[1]), np.float32)
    for g in range(weight.shape[0]):
        s, e = int(edges[g]), int(edges[g + 1])
        out[s:e] = x[s:e] @ weight[g].T
    return out


if __name__ == "__main__":
    # quick host-side layout self-test against numpy (no device)
    rng = np.random.default_rng(0)
    x = rng.standard_normal((NTOK, DIN), dtype=np.float32)
    w = (rng.random((G, DOUT, DIN), dtype=np.float32) - 0.5) / 16
    gl = np.array([223, 557, 1028, 493, 2241, 6807, 73, 3242, 344, 399,
                   222, 11985, 690, 1379, 557, 2528], np.int32)
    tiles, T = _plan(gl)
    print(f"tiles={len([t for t in tiles if t[2] > 0])} padded={len(tiles)} T={T}")
    in_maps = _prep_inputs(x, w, tiles, T)
    # emulate the device program in numpy (bf16 inputs, fp32 accum)
    results = []
    for c in range(NCORES):
        xtc = in_maps[c]["xt"].astype(np.float32)  # [T,128,KT,TT]
        wtc = in_maps[c]["wt"].astype(np.float32)  # [T,128,KT,DOUT]
        otc = np.einsum("tpkn,tpko->tjn...", xtc, wtc) if False else None
        # ot[t, j, ob, n] = sum_{k,p} wt[t,p,k,ob*128+j] * xt[t,p,k,n]
        full = np.einsum("tpko,tpkn->ton", wtc, xtc)  # [T, DOUT, TT]
        results.append({"ot": full.reshape(T, OB, 128, TT).transpose(0, 2, 1, 3)})
    out = _gather_out(results, tiles, T)
    ref = _numpy_ref(x, w, gl)
    err = np.abs(out - ref).max() / np.abs(ref).max()
    print(f"host-layout selftest rel-absmax err (bf16 emulation): {err:.3e}")
    assert err < 2e-2, err
    print("SELFTEST OK")
